# revision 1
# baseline (speedup 1.0000x reference)
"""Trainium2 Bass kernel for nn_AttPool (4-layer GNN + additive-attention pooling).

Strategy (data-parallel over graphs, 32 graphs per NeuronCore):
  * Host re-lays-out the edge list as a per-graph dense normalized adjacency
    Ahat^T = ((A + I) / deg)^T  (pure input encoding; all FLOPs on feature
    data happen on-device).
  * Device, graphs processed in software-pipelined pairs; per graph g:
      - aggT  = sum_c h_block_c^T @ Ahat^T_chunk_c       (PE, [feat, nodes])
      - lin   = aggT_block^T @ W_l  (normal layout)      -> h_next = tanh(lin)
      - linT  = W_l^T @ aggT        (transposed layout)  -> hT_l   = tanh(linT)
        (h feeds the next conv layer and stays in SBUF as cat; hT feeds
        attention; zero transposes anywhere in the conv chain)
      - uT_m  = sum_lc attW[lc,m]^T @ hT_lc ; t = tanh(uT + att_b)
      - s     = sum_m attv_m^T @ t_m                     ([1, 512] scores)
      - per-graph softmax (max via DVE, exp+sum via one ACT op), attention
        column extraction via K=1 matmuls, pooling matmuls, all pipelined
        inside the main loop (no cross-graph barrier).
  * Tiny epilogue: transpose pooled rows, output head matmul, ReLU.
  All matmuls use bf16 operands with fp32 PSUM accumulation (measured
  end-to-end rel-err vs fp32 reference: ~4.5e-3).
"""

import numpy as np
import ml_dtypes

B, N, F = 256, 512, 128
NL = 4
D = 512
OUT = 128
NCORES = 8
GPC = B // NCORES  # graphs per core

BF16 = ml_dtypes.bfloat16

_NC_CACHE = {}


def _build_nc(has_conv_b, has_att_b, has_out_b):
    key = (has_conv_b, has_att_b, has_out_b)
    if key in _NC_CACHE:
        return _NC_CACHE[key]

    import concourse.bacc as bacc
    import concourse.tile as tile
    import concourse.mybir as mybir
    from concourse.masks import make_identity

    f32 = mybir.dt.float32
    bf16 = mybir.dt.bfloat16

    nc = bacc.Bacc(None, target_bir_lowering=False)

    at_d = nc.dram_tensor("at", [GPC, 128, 4 * D], bf16, kind="ExternalInput")
    h0_d = nc.dram_tensor("h0", [GPC, 128, 4 * F], bf16, kind="ExternalInput")
    convw_d = nc.dram_tensor("convw", [128, NL * F], bf16, kind="ExternalInput")
    attw_d = nc.dram_tensor("attw", [128, 4 * D], bf16, kind="ExternalInput")
    attv_d = nc.dram_tensor("attv", [128, 4], bf16, kind="ExternalInput")
    outw_d = nc.dram_tensor("outw", [128, 4 * OUT], bf16, kind="ExternalInput")
    out_d = nc.dram_tensor("out", [GPC, OUT], f32, kind="ExternalOutput")
    convb_d = recip_d = attb_d = outb_d = None
    if has_conv_b:
        convb_d = nc.dram_tensor("convb", [1, NL * F], f32, kind="ExternalInput")
        recip_d = nc.dram_tensor("recipdeg", [GPC, D], f32, kind="ExternalInput")
    if has_att_b:
        attb_d = nc.dram_tensor("attb", [128, 4], f32, kind="ExternalInput")
    if has_out_b:
        outb_d = nc.dram_tensor("outb", [1, OUT], f32, kind="ExternalInput")

    with tile.TileContext(nc) as tc:
        with (
            tc.tile_pool(name="singles", bufs=1) as singles,
            tc.tile_pool(name="dram", bufs=1, space="DRAM") as dram,
        ):
            convw_sb = singles.tile([128, NL * F], bf16)
            attw_sb = singles.tile([128, 4 * D], bf16)
            attv_sb = singles.tile([128, 4], bf16)
            outw_sb = singles.tile([128, 4 * OUT], bf16)
            ident = singles.tile([32, 32], bf16)
            make_identity(nc, ident[:])
            one1 = singles.tile([1, 1], bf16)
            nc.vector.memset(one1[:], 1.0)
            ones128 = singles.tile([128, 1], bf16)
            nc.vector.memset(ones128[:], 1.0)
            convb_sb = attb_sb = outb_sb = ones_sb = None
            if has_conv_b:
                convb_sb = singles.tile([1, NL * F], f32)
                nc.sync.dma_start(convb_sb[:], convb_d[:])
            if has_att_b:
                attb_sb = singles.tile([128, 4], f32)
                nc.sync.dma_start(attb_sb[:], attb_d[:])
            if has_out_b:
                outb_sb = singles.tile([1, OUT], f32)
                nc.sync.dma_start(outb_sb[:], outb_d[:])
                ones_sb = singles.tile([1, 32], f32)
                nc.vector.memset(ones_sb[:], 1.0)

            pstack = singles.tile([GPC, D], bf16)

            # ---------------- Phase A: convs + attention scores ----------
            cat_tiles = {}
            with (
                tc.tile_pool(name="at", bufs=4) as p_at,
                tc.tile_pool(name="h", bufs=4) as p_h,
                tc.tile_pool(name="cat", bufs=GPC * NL + 1) as p_cat,
                tc.tile_pool(name="aggT", bufs=3) as p_aggT,
                tc.tile_pool(name="hT", bufs=20) as p_hT,
                tc.tile_pool(name="t", bufs=4) as p_t,
                tc.tile_pool(name="tt", bufs=8) as p_tt,
                tc.tile_pool(name="rc", bufs=2) as p_rc,
                tc.tile_pool(name="ps_aggT", bufs=2, space="PSUM") as ps_aggT,
                tc.tile_pool(name="ps_lin", bufs=2, space="PSUM") as ps_lin,
                tc.tile_pool(name="ps_small", bufs=1, space="PSUM") as ps_small,
                tc.tile_pool(name="ps_linT", bufs=1, space="PSUM") as ps_linT,
                tc.tile_pool(name="ps_uT", bufs=2, space="PSUM") as ps_uT,
            ):
                hcur = {}
                hTs = {}
                for gp in range(0, GPC, 2):
                    pair = (gp, gp + 1)
                    at_sbs = {}
                    recips = {}
                    for gg in pair:
                        at_tile = p_at.tile([128, 4 * D], bf16, tag="at")
                        at_sbs[gg] = at_tile
                        h0_tile = p_h.tile([128, 4 * F], bf16, tag="h")
                        hcur[gg] = h0_tile
                        if gg == 0:
                            # split first graph's loads per chunk so the first
                            # matmul starts as soon as chunk 0 lands
                            for c in range(4):
                                nc.sync.dma_start(
                                    h0_tile[:, c * F : (c + 1) * F],
                                    h0_d[gg, :, c * F : (c + 1) * F],
                                )
                                nc.sync.dma_start(
                                    at_tile[:, c * D : (c + 1) * D],
                                    at_d[gg, :, c * D : (c + 1) * D],
                                )
                        else:
                            nc.sync.dma_start(at_tile[:], at_d[gg])
                            nc.sync.dma_start(h0_tile[:], h0_d[gg])
                        hTs[gg] = []
                        if has_conv_b:
                            rc_tile = p_rc.tile([1, D], f32)
                            recips[gg] = rc_tile
                            nc.sync.dma_start(rc_tile[:], recip_d[gg : gg + 1, :])
                    if gp == 0:
                        nc.sync.dma_start(convw_sb[:], convw_d[:])
                        nc.sync.dma_start(attw_sb[:], attw_d[:])
                        nc.sync.dma_start(attv_sb[:], attv_d[:])
                        nc.sync.dma_start(outw_sb[:], outw_d[:])

                    for l in range(NL):
                        aggT_pss = {}
                        for gg in pair:
                            aggT_ps = ps_aggT.tile([128, D], mybir.dt.float32)
                            aggT_pss[gg] = aggT_ps
                            for c in range(4):
                                nc.tensor.matmul(
                                    aggT_ps[:],
                                    hcur[gg][:, c * F : (c + 1) * F],
                                    at_sbs[gg][:, c * D : (c + 1) * D],
                                    start=(c == 0),
                                    stop=(c == 3),
                                )
                        for gg in pair:
                            aggT_sb = p_aggT.tile([128, D], bf16)
                            nc.vector.tensor_copy(aggT_sb[:], aggT_pss[gg][:])

                            lin_ps = ps_lin.tile([128, D], mybir.dt.float32)
                            for r in range(4):
                                o = lin_ps[:, r * F : (r + 1) * F]
                                if has_conv_b:
                                    nc.tensor.matmul(
                                        o,
                                        recips[gg][0:1, r * F : (r + 1) * F],
                                        convb_sb[0:1, l * F : (l + 1) * F],
                                        start=True,
                                        stop=False,
                                    )
                                nc.tensor.matmul(
                                    o,
                                    aggT_sb[:, r * F : (r + 1) * F],
                                    convw_sb[:, l * F : (l + 1) * F],
                                    start=not has_conv_b,
                                    stop=True,
                                )
                            h_next = p_cat.tile([128, 4 * F], bf16, tag="cat")
                            cat_tiles[(gg, l)] = h_next
                            nc.scalar.activation(
                                h_next[:], lin_ps[:], mybir.ActivationFunctionType.Tanh
                            )

                            linT_ps = ps_linT.tile([128, D], mybir.dt.float32)
                            if has_conv_b:
                                nc.tensor.matmul(
                                    linT_ps[:],
                                    convb_sb[0:1, l * F : (l + 1) * F],
                                    recips[gg][0:1, :],
                                    start=True,
                                    stop=False,
                                )
                            nc.tensor.matmul(
                                linT_ps[:],
                                convw_sb[:, l * F : (l + 1) * F],
                                aggT_sb[:],
                                start=not has_conv_b,
                                stop=True,
                            )
                            hT_l = p_hT.tile([128, D], bf16)
                            nc.scalar.activation(
                                hT_l[:], linT_ps[:], mybir.ActivationFunctionType.Tanh
                            )
                            hTs[gg].append(hT_l)
                            hcur[gg] = h_next

                    for gg in pair:
                        s4_ps = ps_small.tile([128, D], mybir.dt.float32, tag="small")
                        nc.vector.memset(s4_ps[:], 0.0)
                        t_sbs = []
                        for m in range(4):
                            uT_ps = ps_uT.tile([128, D], mybir.dt.float32)
                            for lc in range(4):
                                nc.tensor.matmul(
                                    uT_ps[:],
                                    attw_sb[:, lc * D + m * F : lc * D + (m + 1) * F],
                                    hTs[gg][lc][:],
                                    start=(lc == 0),
                                    stop=(lc == 3),
                                )
                            t_sb = p_tt.tile([128, D], bf16, tag="t")
                            bias = attb_sb[:, m : m + 1] if has_att_b else 0.0
                            nc.scalar.activation(
                                t_sb[:],
                                uT_ps[:],
                                mybir.ActivationFunctionType.Tanh,
                                bias=bias,
                            )
                            t_sbs.append(t_sb)
                        # 4 concurrent M=1 matmuls on distinct PE column groups,
                        # emitted back-to-back so they pack in the array;
                        # partial rows land at partitions 32*m
                        for m in range(4):
                            nc.tensor.matmul(
                                s4_ps[32 * m : 32 * m + 1, :],
                                attv_sb[:, m : m + 1],
                                t_sbs[m][:],
                                start=True,
                                stop=True,
                                tile_position=(0, 32 * m),
                            )
                        s4_sb = p_t.tile([128, D], bf16, tag="s4")
                        nc.vector.tensor_copy(s4_sb[:], s4_ps[:])
                        s_ps = ps_small.tile([1, D], mybir.dt.float32, tag="small")
                        nc.tensor.matmul(
                            s_ps[:], ones128[:], s4_sb[:], start=True, stop=True
                        )
                        # per-graph softmax (unnormalized) + attn columns + pooling
                        negmax = p_t.tile([1, 1], mybir.dt.float32, tag="nm")
                        nc.vector.tensor_reduce(
                            negmax[:],
                            s_ps[:],
                            axis=mybir.AxisListType.X,
                            op=mybir.AluOpType.max,
                            negate=True,
                        )
                        attn_u = p_t.tile([1, D], bf16, tag="attnu")
                        ssum = p_t.tile([1, 1], mybir.dt.float32, tag="ssum")
                        nc.scalar.activation(
                            attn_u[:],
                            s_ps[:],
                            mybir.ActivationFunctionType.Exp,
                            bias=negmax[:],
                            accum_out=ssum[:],
                        )
                        recip_s = p_t.tile([1, 1], mybir.dt.float32, tag="rcs")
                        nc.vector.reciprocal(recip_s[:], ssum[:])
                        col_ps = ps_small.tile([128, 4], mybir.dt.float32, tag="small")
                        for r in range(4):
                            nc.tensor.matmul(
                                col_ps[:, r : r + 1],
                                attn_u[0:1, r * 128 : (r + 1) * 128],
                                one1[:],
                                start=(r == 0),
                                stop=(r == 3),
                            )
                        attn_col = p_t.tile([128, 4], bf16, tag="acol")
                        nc.vector.tensor_copy(attn_col[:], col_ps[:])
                        pooled4_ps = ps_small.tile(
                            [128, D], mybir.dt.float32, tag="small"
                        )
                        nc.vector.memset(pooled4_ps[:], 0.0)
                        for l in range(NL):
                            for r in range(4):
                                nc.tensor.matmul(
                                    pooled4_ps[32 * r : 32 * r + 1, l * F : (l + 1) * F],
                                    attn_col[:, r : r + 1],
                                    cat_tiles[(gg, l)][:, r * F : (r + 1) * F],
                                    start=(l == 0),
                                    stop=(l == 3),
                                    tile_position=(0, 32 * r),
                                )
                        pooled4_sb = p_t.tile([128, D], bf16, tag="s4")
                        nc.vector.tensor_copy(pooled4_sb[:], pooled4_ps[:])
                        pooled_ps = ps_small.tile([1, D], mybir.dt.float32, tag="small")
                        nc.tensor.matmul(
                            pooled_ps[:], ones128[:], pooled4_sb[:], start=True, stop=True
                        )
                        po_sb = p_t.tile([1, D], bf16, tag="po")
                        nc.vector.tensor_scalar_mul(po_sb[:], pooled_ps[:], recip_s[:])
                        nc.sync.dma_start(pstack[gg : gg + 1, :], po_sb[:])

            # ---------------- Phase B: output head --------------------
            with (
                tc.tile_pool(name="fin", bufs=2) as p_fin,
                tc.tile_pool(name="ps_tp", bufs=2, space="PSUM") as ps_tp,
                tc.tile_pool(name="ps_out", bufs=1, space="PSUM") as ps_out,
            ):
                pT = p_fin.tile([128, 4 * GPC], bf16)
                for c in range(4):
                    tp_ps = ps_tp.tile([128, GPC], bf16)
                    nc.tensor.transpose(
                        tp_ps[:], pstack[:, c * 128 : (c + 1) * 128], ident[:]
                    )
                    nc.scalar.copy(pT[:, c * GPC : (c + 1) * GPC], tp_ps[:])
                out_ps = ps_out.tile([GPC, OUT], mybir.dt.float32)
                if has_out_b:
                    nc.tensor.matmul(
                        out_ps[:], ones_sb[:], outb_sb[:], start=True, stop=False
                    )
                for c in range(4):
                    nc.tensor.matmul(
                        out_ps[:],
                        pT[:, c * GPC : (c + 1) * GPC],
                        outw_sb[:, c * OUT : (c + 1) * OUT],
                        start=(c == 0 and not has_out_b),
                        stop=(c == 3),
                    )
                out_sb = p_fin.tile([GPC, OUT], mybir.dt.float32)
                nc.scalar.activation(
                    out_sb[:], out_ps[:], mybir.ActivationFunctionType.Relu
                )
                nc.sync.dma_start(out_d[:], out_sb[:])

    nc.compile()
    _NC_CACHE[key] = nc
    return nc


def _prep_inputs(node_feat, edge_src, edge_dst, conv_W, att_W, att_v, out_W):
    src = edge_src.astype(np.int64)
    dst = edge_dst.astype(np.int64)
    ls = src - (dst // N) * N  # src local id within dst's graph
    idx = dst * N + ls
    counts = np.bincount(idx, minlength=B * N * N).astype(np.float32)
    A = counts.reshape(B, N, N)
    iN = np.arange(N)
    A[:, iN, iN] += 1.0
    degs = A.sum(axis=2)  # == deg + 1
    Ahat = A / degs[:, :, None]
    At = np.ascontiguousarray(Ahat.transpose(0, 2, 1))  # [g, src, dst]
    at_host = np.ascontiguousarray(
        At.reshape(B, 4, 128, N).transpose(0, 2, 1, 3)
    ).reshape(B, 128, 4 * N)

    h0_host = np.ascontiguousarray(
        node_feat.reshape(B, 4, 128, F).transpose(0, 2, 1, 3)
    ).reshape(B, 128, 4 * F)

    convw2 = np.ascontiguousarray(conv_W.transpose(1, 0, 2)).reshape(128, NL * F)
    attw2 = np.ascontiguousarray(
        att_W.reshape(4, 128, D).transpose(1, 0, 2)
    ).reshape(128, 4 * D)
    attv2 = np.ascontiguousarray(att_v.reshape(4, 128).T)
    outw2 = np.ascontiguousarray(
        out_W.reshape(4, 128, OUT).transpose(1, 0, 2)
    ).reshape(128, 4 * OUT)
    return at_host, h0_host, convw2, attw2, attv2, outw2, degs


def kernel(
    node_feat,
    edge_src,
    edge_dst,
    conv_W,
    conv_b,
    att_W,
    att_b,
    att_v,
    out_W,
    out_b,
):
    from concourse.bass_utils import run_bass_kernel_spmd

    at_host, h0_host, convw2, attw2, attv2, outw2, degs = _prep_inputs(
        np.asarray(node_feat, dtype=np.float32),
        np.asarray(edge_src),
        np.asarray(edge_dst),
        np.asarray(conv_W, dtype=np.float32),
        np.asarray(att_W, dtype=np.float32),
        np.asarray(att_v, dtype=np.float32),
        np.asarray(out_W, dtype=np.float32),
    )
    conv_b = np.asarray(conv_b, dtype=np.float32)
    att_b = np.asarray(att_b, dtype=np.float32)
    out_b = np.asarray(out_b, dtype=np.float32)
    has_conv_b = bool(np.any(conv_b))
    has_att_b = bool(np.any(att_b))
    has_out_b = bool(np.any(out_b))

    nc = _build_nc(has_conv_b, has_att_b, has_out_b)

    convw_b = convw2.astype(BF16)
    attw_b = attw2.astype(BF16)
    attv_b = attv2.astype(BF16)
    outw_b = outw2.astype(BF16)
    attb2 = np.ascontiguousarray(att_b.reshape(4, 128).T) if has_att_b else None

    in_maps = []
    for c in range(NCORES):
        sl = slice(c * GPC, (c + 1) * GPC)
        m = {
            "at": at_host[sl].astype(BF16),
            "h0": h0_host[sl].astype(BF16),
            "convw": convw_b,
            "attw": attw_b,
            "attv": attv_b,
            "outw": outw_b,
        }
        if has_conv_b:
            m["convb"] = conv_b
            m["recipdeg"] = (1.0 / degs[sl]).astype(np.float32)
        if has_att_b:
            m["attb"] = attb2
        if has_out_b:
            m["outb"] = out_b.reshape(1, OUT)
        in_maps.append(m)

    res = run_bass_kernel_spmd(nc, in_maps, core_ids=list(range(NCORES)))
    out = np.concatenate([r["out"] for r in res.results], axis=0)
    return np.ascontiguousarray(out.astype(np.float32))

